# revision 8
# baseline (speedup 1.0000x reference)
"""Multi-head causal attention (nn_MultiHeadAttention) on 8 trn2 NeuronCores.

Sharding: 8 cores = 2 batches x 4 head-groups (4 heads each). Each core:
  - QKV projection for its batch/head-group (x fed pre-transposed [C,T] so
    every matmul contracts over the partition axis with natural DMA layouts)
  - causal attention for its 4 heads (flash-style tiling, no max-subtraction:
    scores are ~N(0,1) so exp never overflows; softmax denominator comes from
    a DVE accumulation + one ones-matmul that also broadcasts it across
    partitions)
  - partial output projection (its heads' rows of W_proj)
Host side: shards inputs, sums the 4 partials per batch, adds b_proj.

Matmuls run in float32r (full PE rate at moving-dim 512, vs 4x slower fp32).
The PV pair (V tiles + exp(P) tiles) optionally drops to bf16 (V_BF16) to
save SBUF. The mask input is analyzed on the host: attention tiles are
classified full/empty/mixed, empty tiles are skipped, mixed tiles get a
multiplicative {0,1} pattern (4 unique patterns for a causal mask) applied by
GPSIMD. This specializes the compiled kernel to the actual mask contents.
"""

import hashlib
import numpy as np

_B, _C, _H = 2, 2048, 16
_HD = 128
_NG = 4                  # head groups (cores per batch)
_HL = _H // _NG          # heads per core
_DL = _HL * _HD          # local head dims per core
_P = 128
_QR = 512                # q-range / moving free dim
_NCORES = 8
_SCALE = float(_HD) ** -0.5

V_BF16 = True            # PV-pair dtype: True = bf16 V/P tiles, False = fp32r

_BUILD_CACHE = {}


# ---------------------------------------------------------------------------
# Workarounds for this container's walrus build (max 1 sync-wait per
# instruction, 2 for EventSemaphore).
# ---------------------------------------------------------------------------

def _patch_concourse():
    import concourse.tile as tile

    if getattr(tile.TileContext, "_ant_waitfix", False):
        return

    def _patched(self, tick_clock, wait_clock):
        drain_inst = self.nc.sync.drain()
        wait_clock.add_sem_waits(
            drain_inst.ins, tile.ScopedClock({None: tick_clock.global_clock})
        )
        self.nc.all_engine_barrier()
        popped = self.nc._tile_sem_poison_stack.pop()
        assert popped is self._sem_poison
        self.nc.clear_and_free_semaphores(list(self.sems.allocated().values()))

    tile.TileContext._drain_and_barrier = _patched
    tile.TileContext._ant_waitfix = True


def _split_excess_waits(nc):
    """Move excess sync-waits onto same-engine NoOps inserted just before."""
    from concourse import mybir
    import bass_rust as _br

    n_split = 0
    for fn in nc.m.functions:
        for blk in fn.blocks:
            out = []
            for inst in blk.instructions:
                si = inst.sync_info
                cap = 2 if isinstance(inst, mybir.InstEventSemaphore) else 1
                if si is not None and si.on_wait is not None and len(si.on_wait) > cap:
                    waits = list(si.on_wait)
                    extra, keep = waits[:-cap], waits[-cap:]
                    for j, w in enumerate(extra):
                        nop = mybir.InstNoOp(name=f"{inst.name}-ws{j}", ins=[], outs=[])
                        nop.engine = inst.engine
                        nop.sync_info = _br.SyncInfo(on_wait=[w], on_update=[])
                        out.append(nop)
                        n_split += 1
                    inst.sync_info = _br.SyncInfo(
                        on_wait=keep, on_update=list(si.on_update)
                    )
                out.append(inst)
            blk.instructions = out
    return n_split


# ---------------------------------------------------------------------------
# Mask analysis (host): classify [QR x 128] attention tiles.
# ---------------------------------------------------------------------------

def _analyze_mask(m2, T_):
    import ml_dtypes

    NQR = T_ // _QR
    NKT = T_ // _P
    plan = {}
    pats = []
    pat_idx = {}
    for qr in range(NQR):
        for kt in range(NKT):
            sub = m2[qr * _QR:(qr + 1) * _QR, kt * _P:(kt + 1) * _P]
            if sub.all():
                plan[(qr, kt)] = ("full", -1)
            elif not sub.any():
                plan[(qr, kt)] = ("empty", -1)
            else:
                p = np.ascontiguousarray(sub.T).astype(ml_dtypes.bfloat16)
                key = p.tobytes()
                if key not in pat_idx:
                    pat_idx[key] = len(pats)
                    pats.append(p)
                plan[(qr, kt)] = ("mixed", pat_idx[key])
    if pats:
        patterns = np.stack(pats)
    else:
        patterns = np.zeros((1, _P, _QR), ml_dtypes.bfloat16)
    return plan, patterns


# ---------------------------------------------------------------------------
# Kernel builder
# ---------------------------------------------------------------------------

def _build_nc(T_, plan, n_pat, v_bf16, split_waits=True, repeat=1, phases="ABC"):
    import concourse.bass as bass
    import concourse.tile as tile
    from concourse import mybir

    _patch_concourse()

    F = mybir.dt.float32
    R = mybir.dt.float32r
    BF = mybir.dt.bfloat16
    VDT = BF if v_bf16 else R     # V-store / P-tile dtype (PV matmul pair)
    TDT = BF if v_bf16 else F     # V-transpose staging dtype
    Exp = mybir.ActivationFunctionType.Exp

    NKT = T_ // _P
    NQR = T_ // _QR
    TRW = 1024 if T_ % 1024 == 0 else _QR
    NTR = T_ // TRW
    SUB = TRW // _QR
    CT = _C // _P
    NF = 3 * _HL

    nc = bass.Bass(trn_type="TRN2", target_bir_lowering=False, debug=False)
    xt = nc.dram_tensor("xt", [_C, T_], F, kind="ExternalInput")
    wqkv = nc.dram_tensor("wqkv", [_C, 3 * _DL], F, kind="ExternalInput")
    wproj = nc.dram_tensor("wproj", [_DL, _C], F, kind="ExternalInput")
    mpat = nc.dram_tensor("mpat", [max(n_pat, 1), _P, _QR], BF, kind="ExternalInput")
    aux = nc.dram_tensor("aux", [_P, 2 * _P], F, kind="ExternalInput")
    out = nc.dram_tensor("out", [T_, _C], F, kind="ExternalOutput")

    wq3 = wqkv.ap().rearrange("(ct ci) f -> ci ct f", ci=_P)

    with tile.TileContext(nc) as tc:
        with (
            tc.tile_pool(name="const", bufs=1) as cpool,
            tc.tile_pool(name="xw", bufs=1) as xwpool,
            tc.tile_pool(name="wst", bufs=2) as wstpool,
            tc.tile_pool(name="store", bufs=1) as stpool,
            tc.tile_pool(name="qt", bufs=8) as qtpool,
            tc.tile_pool(name="vstage", bufs=2) as vspool,
            tc.tile_pool(name="pt", bufs=5) as ptpool,
            tc.tile_pool(name="sacc", bufs=2) as sapool,
            tc.tile_pool(name="ysb", bufs=6) as ypool,
            tc.tile_pool(name="wp", bufs=6) as wppool,
            tc.tile_pool(name="osb", bufs=3) as opool,
            tc.tile_pool(name="mm_ps", bufs=2, space="PSUM") as mmps,
            tc.tile_pool(name="t_ps", bufs=2, space="PSUM") as tps,
            tc.tile_pool(name="att_ps", bufs=2, space="PSUM") as attps,
            tc.tile_pool(name="y_ps", bufs=2, space="PSUM") as yps,
        ):
            # constants
            idt = cpool.tile([_P, _P], F, name="idt")
            nc.sync.dma_start(idt[:], aux.ap()[:, 0:_P])
            if v_bf16:
                idt_bf = cpool.tile([_P, _P], BF, name="idt_bf")
                nc.vector.tensor_copy(idt_bf[:], idt[:])
                idt_t = idt_bf
            else:
                idt_t = idt
            ones_r = cpool.tile([_P, _P], R, name="ones_r")
            nc.sync.dma_start(ones_r[:], aux.ap()[:, _P:2 * _P].bitcast(R))

            resident_masks = n_pat <= 24
            mtiles = []
            if resident_masks:
                for i in range(n_pat):
                    mt = cpool.tile([_P, _QR], BF, name=f"mask{i}")
                    nc.sync.dma_start(mt[:], mpat.ap()[i])
                    mtiles.append(mt)

            # persistent stores
            KT = [stpool.tile([_P, T_], R, name=f"ktile{h}") for h in range(_HL)]
            V = [stpool.tile([_P, _DL], VDT, name=f"vtile{t}") for t in range(NKT)]
            qt_tiles = {}
            y_tiles = {}

            def A_phase(tr):
                xw = []
                for c in range(CT):
                    t = xwpool.tile([_P, TRW], R, name=f"xw{c}")
                    nc.sync.dma_start(
                        t[:],
                        xt.ap()[c * _P:(c + 1) * _P, tr * TRW:(tr + 1) * TRW].bitcast(R),
                    )
                    xw.append(t)
                for f in range(NF):
                    kind, h = f // _HL, f % _HL
                    wt = wstpool.tile([_P, CT, _P], R, name="wt")
                    col = kind * _DL + h * _HD
                    nc.sync.dma_start(wt[:], wq3[:, :, col:col + _HD].bitcast(R))
                    for sub in range(SUB):
                        qr = tr * SUB + sub
                        ps = mmps.tile([_P, _QR], F, name="mmps")
                        for c in range(CT):
                            nc.tensor.matmul(
                                ps[:],
                                lhsT=wt[:, c, :],
                                rhs=xw[c][:, sub * _QR:(sub + 1) * _QR],
                                start=(c == 0),
                                stop=(c == CT - 1),
                            )
                        if kind == 0:
                            q = qtpool.tile([_P, _QR], R, name="qt")
                            nc.any.tensor_copy(q[:], ps[:])
                            qt_tiles[(h, qr)] = q
                        elif kind == 1:
                            nc.any.tensor_copy(KT[h][:, qr * _QR:(qr + 1) * _QR], ps[:])
                        else:
                            vs = vspool.tile([_P, _QR], TDT, name="vst")
                            nc.any.tensor_copy(vs[:], ps[:])
                            pt_ = tps.tile([_P, _QR], TDT, name="tps")
                            for j in range(_QR // _P):
                                nc.tensor.transpose(
                                    pt_[:, j * _P:(j + 1) * _P],
                                    vs[:, j * _P:(j + 1) * _P],
                                    idt_t[:],
                                )
                            for j in range(_QR // _P):
                                tt = qr * (_QR // _P) + j
                                nc.any.tensor_copy(
                                    V[tt][:, h * _HD:(h + 1) * _HD],
                                    pt_[:, j * _P:(j + 1) * _P],
                                )

            def get_mask_tile(pidx):
                if resident_masks:
                    return mtiles[pidx]
                mt = ptpool.tile([_P, _QR], BF, name="mstream", bufs=4)
                nc.sync.dma_start(mt[:], mpat.ap()[pidx])
                return mt

            def B_phase(h, qr):
                kts = [
                    kt for kt in range(NKT)
                    if plan.get((qr, kt), ("full", -1))[0] != "empty"
                ]
                ysb = ypool.tile([_P, _QR], R, name="ysb")
                if not kts:
                    nc.vector.memset(ysb[:], 0.0)
                    y_tiles[(h, qr)] = ysb
                    return
                ps_y = yps.tile([_P, _QR], F, name="yps")
                sacc = sapool.tile([_P, _QR], R, name="sacc")
                qtile = qt_tiles[(h, qr)]
                for i, kt in enumerate(kts):
                    ps_s = attps.tile([_P, _QR], F, name="sps")
                    nc.tensor.matmul(
                        ps_s[:],
                        lhsT=KT[h][:, kt * _P:(kt + 1) * _P],
                        rhs=qtile[:],
                        start=True,
                        stop=True,
                    )
                    pt = ptpool.tile([_P, _QR], VDT, name="pt")
                    nc.scalar.activation(pt[:], ps_s[:], Exp, scale=_SCALE)
                    cat, pidx = plan.get((qr, kt), ("full", -1))
                    if cat == "mixed":
                        nc.gpsimd.tensor_mul(pt[:], pt[:], get_mask_tile(pidx)[:])
                    if i == 0:
                        nc.vector.tensor_copy(sacc[:], pt[:])
                    else:
                        nc.vector.tensor_add(sacc[:], sacc[:], pt[:])
                    nc.tensor.matmul(
                        ps_y[:],
                        lhsT=V[kt][:, h * _HD:(h + 1) * _HD],
                        rhs=pt[:],
                        start=(i == 0),
                        stop=(i == len(kts) - 1),
                    )
                ps_bc = attps.tile([_P, _QR], F, name="sps")
                nc.tensor.matmul(
                    ps_bc[:], lhsT=ones_r[:], rhs=sacc[:], start=True, stop=True
                )
                rec = sapool.tile([_P, _QR], F, name="rec")
                nc.vector.reciprocal(rec[:], ps_bc[:])
                nc.vector.tensor_mul(ysb[:], ps_y[:], rec[:])
                y_tiles[(h, qr)] = ysb

            def C_phase(qr):
                for fr in range(_C // _QR):
                    wps = []
                    for h in range(_HL):
                        w = wppool.tile([_P, _QR], R, name="wp")
                        nc.sync.dma_start(
                            w[:],
                            wproj.ap()[
                                h * _HD:(h + 1) * _HD, fr * _QR:(fr + 1) * _QR
                            ].bitcast(R),
                        )
                        wps.append(w)
                    for tsub in range(_QR // _P):
                        ps = mmps.tile([_P, _QR], F, name="mmps")
                        for h in range(_HL):
                            nc.tensor.matmul(
                                ps[:],
                                lhsT=y_tiles[(h, qr)][:, tsub * _P:(tsub + 1) * _P],
                                rhs=wps[h][:],
                                start=(h == 0),
                                stop=(h == _HL - 1),
                            )
                        ot = opool.tile([_P, _QR], F, name="osb")
                        nc.any.tensor_copy(ot[:], ps[:])
                        t0 = qr * _QR + tsub * _P
                        nc.sync.dma_start(
                            out.ap()[t0:t0 + _P, fr * _QR:(fr + 1) * _QR], ot[:]
                        )

            for _rep in range(repeat):
                for tr in range(NTR):
                    A_phase(tr)
                    for sub in range(SUB):
                        qr = tr * SUB + sub
                        if "B" in phases:
                            for h in range(_HL):
                                B_phase(h, qr)
                            if "C" in phases:
                                C_phase(qr)

    if split_waits:
        _split_excess_waits(nc)
    return nc


# ---------------------------------------------------------------------------
# Execution via PJRT (axon tunnel)
# ---------------------------------------------------------------------------

def _make_runner(nc, n_cores):
    import jax
    from concourse import bass2jax, mybir
    from jax.sharding import Mesh, PartitionSpec

    try:
        from jax.experimental.shard_map import shard_map
    except ImportError:
        from jax import shard_map  # newer jax

    bass2jax.install_neuronx_cc_hook()

    partition_name = nc.partition_id_tensor.name if nc.partition_id_tensor else None
    in_names, out_names, out_avals, zero_outs = [], [], [], []
    for alloc in nc.m.functions[0].allocations:
        if not isinstance(alloc, mybir.MemoryLocationSet):
            continue
        name = alloc.memorylocations[0].name
        if alloc.kind == "ExternalInput":
            if name != partition_name:
                in_names.append(name)
        elif alloc.kind == "ExternalOutput":
            out_names.append(name)
            shape = tuple(alloc.tensor_shape)
            dtype = mybir.dt.np(alloc.dtype)
            out_avals.append(jax.core.ShapedArray(shape, dtype))
            zero_outs.append(np.zeros(shape, dtype))
    n_params = len(in_names)
    all_names = list(in_names) + list(out_names)
    if partition_name is not None:
        all_names.append(partition_name)
    all_names = tuple(all_names)

    def _body(*args):
        operands = list(args)
        if partition_name is not None:
            operands.append(bass2jax.partition_id_tensor())
        outs = bass2jax._bass_exec_p.bind(
            *operands,
            out_avals=tuple(out_avals),
            in_names=all_names,
            out_names=tuple(out_names),
            lowering_input_output_aliases=(),
            sim_require_finite=True,
            sim_require_nnan=True,
            nc=nc,
        )
        return tuple(outs)

    devices = jax.devices()[:n_cores]
    mesh = Mesh(np.asarray(devices), ("core",))
    in_specs = (PartitionSpec("core"),) * (n_params + len(out_names))
    out_specs = (PartitionSpec("core"),) * len(out_names)
    donate = tuple(range(n_params, n_params + len(out_names)))
    fn = jax.jit(
        shard_map(
            _body, mesh=mesh, in_specs=in_specs, out_specs=out_specs, check_rep=False
        ),
        donate_argnums=donate,
        keep_unused=True,
    )
    return {
        "fn": fn,
        "mesh": mesh,
        "in_names": in_names,
        "out_names": out_names,
        "out_avals": out_avals,
        "zero_outs": zero_outs,
        "n_cores": n_cores,
    }


def _concat_inputs(runner, per_core_inmaps):
    return [
        np.concatenate([np.asarray(m[name]) for m in per_core_inmaps], axis=0)
        for name in runner["in_names"]
    ]


def _concat_zeros(runner):
    n = runner["n_cores"]
    return [
        np.zeros((n * z.shape[0], *z.shape[1:]), z.dtype) for z in runner["zero_outs"]
    ]


def _run_cores(runner, per_core_inmaps):
    outs = runner["fn"](*_concat_inputs(runner, per_core_inmaps), *_concat_zeros(runner))
    n = runner["n_cores"]
    res = [np.asarray(o) for o in outs]
    return [
        {
            name: res[i].reshape(n, *runner["out_avals"][i].shape)[c]
            for i, name in enumerate(runner["out_names"])
        }
        for c in range(n)
    ]


# ---------------------------------------------------------------------------
# Public entry point
# ---------------------------------------------------------------------------

def _get_runner(T_, plan, patterns):
    key = (
        T_,
        V_BF16,
        tuple(sorted((k, v[0], v[1]) for k, v in plan.items())),
        hashlib.sha1(patterns.tobytes()).hexdigest(),
    )
    if key not in _BUILD_CACHE:
        nc = _build_nc(T_, plan, patterns.shape[0], V_BF16)
        _BUILD_CACHE[key] = _make_runner(nc, _NCORES)
    return _BUILD_CACHE[key]


def _make_inmaps(x, mask_plan_patterns, W_qkv, W_proj):
    plan, patterns = mask_plan_patterns
    Bn, T_, C_ = x.shape
    aux = np.concatenate(
        [np.eye(_P, dtype=np.float32), np.ones((_P, _P), np.float32)], axis=1
    )
    in_maps = []
    xts = [np.ascontiguousarray(x[b].T) for b in range(Bn)]
    for core in range(_NCORES):
        b, g = divmod(core, _NG)
        wqkv_g = np.ascontiguousarray(
            np.concatenate(
                [
                    W_qkv[:, g * _DL:(g + 1) * _DL],
                    W_qkv[:, C_ + g * _DL:C_ + (g + 1) * _DL],
                    W_qkv[:, 2 * C_ + g * _DL:2 * C_ + (g + 1) * _DL],
                ],
                axis=1,
            )
        )
        wproj_g = np.ascontiguousarray(W_proj[g * _DL:(g + 1) * _DL, :])
        in_maps.append(
            {
                "xt": xts[b],
                "wqkv": wqkv_g,
                "wproj": wproj_g,
                "mpat": patterns,
                "aux": aux,
            }
        )
    return in_maps


def _combine(per_core, b_proj, Bn, T_, C_):
    outp = np.empty((Bn, T_, C_), np.float32)
    for b in range(Bn):
        acc = per_core[b * _NG]["out"].astype(np.float32, copy=True)
        for g in range(1, _NG):
            acc += per_core[b * _NG + g]["out"]
        outp[b] = acc + b_proj[None, :].astype(np.float32)
    return outp


def kernel(**inputs):
    x = np.asarray(inputs["x"], dtype=np.float32)
    Bn, T_, C_ = x.shape
    assert (Bn, C_) == (_B, _C), f"kernel hardcoded for B={_B}, C={_C}"
    mask = np.asarray(inputs["mask"]).astype(bool).reshape(T_, T_)
    W_qkv = np.asarray(inputs["W_qkv"], dtype=np.float32)
    W_proj = np.asarray(inputs["W_proj"], dtype=np.float32)
    b_proj = np.asarray(inputs["b_proj"], dtype=np.float32)

    plan, patterns = _analyze_mask(mask, T_)
    runner = _get_runner(T_, plan, patterns)
    in_maps = _make_inmaps(x, (plan, patterns), W_qkv, W_proj)
    per_core = _run_cores(runner, in_maps)
    return _combine(per_core, b_proj, Bn, T_, C_)


# revision 25
# speedup vs baseline: 1.2360x; 1.2360x over previous
"""Multi-head causal attention (nn_MultiHeadAttention) on 8 trn2 NeuronCores.

Sharding: 8 cores = 2 batches x 4 head-groups (4 heads each). Each core:
  - QKV projection for its batch/head-group (x fed pre-transposed [C,T] so
    every matmul contracts over the partition axis with natural DMA layouts)
  - causal attention for its 4 heads (flash-style tiling, no max-subtraction:
    scores are ~N(0,1) so exp never overflows; softmax denominator comes from
    a DVE accumulation + one ones-matmul that also broadcasts it across
    partitions)
  - partial output projection (its heads' rows of W_proj)
Host side: shards inputs, sums the 4 partials per batch, adds b_proj.

Matmuls run in float32r (full PE rate at moving-dim 512, vs 4x slower fp32).
The PV pair (V tiles + exp(P) tiles) optionally drops to bf16 (V_BF16) to
save SBUF. The mask input is analyzed on the host: attention tiles are
classified full/empty/mixed, empty tiles are skipped, mixed tiles get a
multiplicative {0,1} pattern (4 unique patterns for a causal mask) applied by
GPSIMD. This specializes the compiled kernel to the actual mask contents.
"""

import hashlib
import numpy as np

_B, _C, _H = 2, 2048, 16
_HD = 128
_NG = 4                  # head groups (cores per batch)
_HL = _H // _NG          # heads per core
_DL = _HL * _HD          # local head dims per core
_P = 128
_QR = 512                # q-range / moving free dim
_NCORES = 8
_SCALE = float(_HD) ** -0.5

V_BF16 = True            # PV-pair dtype: True = bf16 V/P tiles, False = fp32r

_BUILD_CACHE = {}


# ---------------------------------------------------------------------------
# Workarounds for this container's walrus build (max 1 sync-wait per
# instruction, 2 for EventSemaphore).
# ---------------------------------------------------------------------------

def _patch_concourse():
    import concourse.tile as tile

    if getattr(tile.TileContext, "_ant_waitfix", False):
        return

    def _patched(self, tick_clock, wait_clock):
        drain_inst = self.nc.sync.drain()
        wait_clock.add_sem_waits(
            drain_inst.ins, tile.ScopedClock({None: tick_clock.global_clock})
        )
        self.nc.all_engine_barrier()
        popped = self.nc._tile_sem_poison_stack.pop()
        assert popped is self._sem_poison
        self.nc.clear_and_free_semaphores(list(self.sems.allocated().values()))

    tile.TileContext._drain_and_barrier = _patched
    tile.TileContext._ant_waitfix = True


def _split_excess_waits(nc):
    """Move excess sync-waits onto same-engine NoOps inserted just before."""
    from concourse import mybir
    import bass_rust as _br

    n_split = 0
    for fn in nc.m.functions:
        for blk in fn.blocks:
            out = []
            for inst in blk.instructions:
                si = inst.sync_info
                cap = 2 if isinstance(inst, mybir.InstEventSemaphore) else 1
                if si is not None and si.on_wait is not None and len(si.on_wait) > cap:
                    waits = list(si.on_wait)
                    extra, keep = waits[:-cap], waits[-cap:]
                    for j, w in enumerate(extra):
                        nop = mybir.InstNoOp(name=f"{inst.name}-ws{j}", ins=[], outs=[])
                        nop.engine = inst.engine
                        nop.sync_info = _br.SyncInfo(on_wait=[w], on_update=[])
                        out.append(nop)
                        n_split += 1
                    inst.sync_info = _br.SyncInfo(
                        on_wait=keep, on_update=list(si.on_update)
                    )
                out.append(inst)
            blk.instructions = out
    return n_split


# ---------------------------------------------------------------------------
# Mask analysis (host): classify [QR x 128] attention tiles.
# ---------------------------------------------------------------------------

def _analyze_mask(m2, T_):
    import ml_dtypes

    NQR = T_ // _QR
    NKT = T_ // _P
    plan = {}
    pats = []
    pat_idx = {}
    for qr in range(NQR):
        for kt in range(NKT):
            sub = m2[qr * _QR:(qr + 1) * _QR, kt * _P:(kt + 1) * _P]
            if sub.all():
                plan[(qr, kt)] = ("full", -1, 0)
            elif not sub.any():
                plan[(qr, kt)] = ("empty", -1, 0)
            else:
                p = np.ascontiguousarray(sub.T).astype(ml_dtypes.bfloat16)
                key = p.tobytes()
                if key not in pat_idx:
                    pat_idx[key] = len(pats)
                    pats.append(p)
                # first q row (column in [k,q] layout) with any allowed k:
                # ops on this tile can skip columns < q_lo entirely
                q_lo = int(np.argmax(sub.any(axis=1)))
                plan[(qr, kt)] = ("mixed", pat_idx[key], q_lo)
    if pats:
        patterns = np.stack(pats)
    else:
        patterns = np.zeros((1, _P, _QR), ml_dtypes.bfloat16)
    return plan, patterns


# ---------------------------------------------------------------------------
# Kernel builder
# ---------------------------------------------------------------------------

def _build_nc(T_, plan, n_pat, v_bf16, split_waits=True, repeat=1, phases="ABC"):
    import concourse.bass as bass
    import concourse.tile as tile
    from concourse import mybir

    _patch_concourse()

    F = mybir.dt.float32
    R = mybir.dt.float32r
    BF = mybir.dt.bfloat16
    VDT = BF if v_bf16 else R     # V-store / P-tile dtype (PV matmul pair)
    TDT = BF if v_bf16 else F     # V-transpose staging dtype
    Exp = mybir.ActivationFunctionType.Exp

    NKT = T_ // _P
    NQR = T_ // _QR
    TRW = 1024 if T_ % 1024 == 0 else _QR
    NTR = T_ // TRW
    SUB = TRW // _QR
    CT = _C // _P
    NF = 3 * _HL

    nc = bass.Bass(trn_type="TRN2", target_bir_lowering=False, debug=False)
    xt = nc.dram_tensor("xt", [_C, T_], F, kind="ExternalInput")
    # host pre-arranges: wqkv[f, ci, ct*128+ff] = W_local[ct*128+ci, f*128+ff]
    # so each weight f-tile loads as one contiguous 8KB-per-partition DMA
    wqkv = nc.dram_tensor("wqkv", [NF, _P, _C], F, kind="ExternalInput")
    wproj = nc.dram_tensor("wproj", [_DL, _C], F, kind="ExternalInput")
    mpat = nc.dram_tensor("mpat", [max(n_pat, 1), _P, _QR], BF, kind="ExternalInput")
    aux = nc.dram_tensor("aux", [_P, 2 * _P], F, kind="ExternalInput")
    out = nc.dram_tensor("out", [T_, _C], F, kind="ExternalOutput")

    with tile.TileContext(nc) as tc:
        with (
            tc.tile_pool(name="const", bufs=1) as cpool,
            tc.tile_pool(name="xw", bufs=1) as xwpool,
            tc.tile_pool(name="wst", bufs=2) as wstpool,
            tc.tile_pool(name="store", bufs=1) as stpool,
            # qt needs TWO token-windows of slots: A(tr+1) allocates its 8
            # while B(tr) consumers are still pending (interleaved emission)
            tc.tile_pool(name="qt", bufs=16 if v_bf16 else 6) as qtpool,
            tc.tile_pool(name="vstage", bufs=2) as vspool,
            tc.tile_pool(name="pt", bufs=4) as ptpool,
            tc.tile_pool(name="sacc", bufs=2) as sapool,
            tc.tile_pool(name="ysb", bufs=5) as ypool,
            tc.tile_pool(name="wp", bufs=5 if v_bf16 else 4) as wppool,
            tc.tile_pool(name="osb", bufs=3 if v_bf16 else 2) as opool,
            tc.tile_pool(name="mm_ps", bufs=2, space="PSUM") as mmps,
            tc.tile_pool(name="t_ps", bufs=2, space="PSUM") as tps,
            tc.tile_pool(name="att_ps", bufs=2, space="PSUM") as attps,
            tc.tile_pool(name="y_ps", bufs=2, space="PSUM") as yps,
        ):
            # constants
            idt = cpool.tile([_P, _P], F, name="idt")
            nc.sync.dma_start(idt[:], aux.ap()[:, 0:_P])
            if v_bf16:
                idt_bf = cpool.tile([_P, _P], BF, name="idt_bf")
                nc.vector.tensor_copy(idt_bf[:], idt[:])
                idt_t = idt_bf
            else:
                idt_t = idt
            ones_r = cpool.tile([_P, _P], R, name="ones_r")
            nc.sync.dma_start(ones_r[:], aux.ap()[:, _P:2 * _P].bitcast(R))

            resident_masks = n_pat <= 24
            mtiles = []
            if resident_masks:
                for i in range(n_pat):
                    mt = cpool.tile([_P, _QR], BF, name=f"mask{i}")
                    nc.sync.dma_start(mt[:], mpat.ap()[i])
                    mtiles.append(mt)

            # persistent stores
            KT = [stpool.tile([_P, T_], R, name=f"ktile{h}") for h in range(_HL)]
            V = [stpool.tile([_P, _DL], VDT, name=f"vtile{t}") for t in range(NKT)]
            qt_tiles = {}
            y_tiles = {}

            xw_store = {}

            def A_xload(tr):
                xw = []
                for c in range(CT):
                    t = xwpool.tile([_P, TRW], R, name=f"xw{c}")
                    nc.sync.dma_start(
                        t[:],
                        xt.ap()[c * _P:(c + 1) * _P, tr * TRW:(tr + 1) * TRW].bitcast(R),
                    )
                    xw.append(t)
                xw_store[tr] = xw

            def A_fblock(tr, f):
                xw = xw_store[tr]
                kind, h = f // _HL, f % _HL
                wt = wstpool.tile([_P, _C], R, name="wt")
                nc.sync.dma_start(wt[:], wqkv.ap()[f].bitcast(R))
                for sub in range(SUB):
                    qr = tr * SUB + sub
                    ps = mmps.tile([_P, _QR], F, name="mmps")
                    for c in range(CT):
                        nc.tensor.matmul(
                            ps[:],
                            lhsT=wt[:, c * _P:(c + 1) * _P],
                            rhs=xw[c][:, sub * _QR:(sub + 1) * _QR],
                            start=(c == 0),
                            stop=(c == CT - 1),
                        )
                    if kind == 0:
                        q = qtpool.tile([_P, _QR], R, name="qt")
                        nc.any.tensor_copy(q[:], ps[:])
                        qt_tiles[(h, qr)] = q
                    elif kind == 1:
                        nc.any.tensor_copy(KT[h][:, qr * _QR:(qr + 1) * _QR], ps[:])
                    else:
                        vs = vspool.tile([_P, _QR], TDT, name="vst")
                        nc.any.tensor_copy(vs[:], ps[:])
                        pt_ = tps.tile([_P, _QR], TDT, name="tps")
                        for j in range(_QR // _P):
                            nc.tensor.transpose(
                                pt_[:, j * _P:(j + 1) * _P],
                                vs[:, j * _P:(j + 1) * _P],
                                idt_t[:],
                            )
                        for j in range(_QR // _P):
                            tt = qr * (_QR // _P) + j
                            nc.any.tensor_copy(
                                V[tt][:, h * _HD:(h + 1) * _HD],
                                pt_[:, j * _P:(j + 1) * _P],
                            )

            def get_mask_tile(pidx):
                if resident_masks:
                    return mtiles[pidx]
                mt = ptpool.tile([_P, _QR], BF, name="mstream", bufs=4)
                nc.sync.dma_start(mt[:], mpat.ap()[pidx])
                return mt

            def B_phase(h, qr, qtile):
                kts = [
                    kt for kt in range(NKT)
                    if plan.get((qr, kt), ("full", -1, 0))[0] != "empty"
                ]
                ysb = ypool.tile([_P, _QR], R, name="ysb")
                if not kts:
                    nc.vector.memset(ysb[:].bitcast(mybir.dt.uint32), 0)
                    y_tiles[(h, qr)] = ysb
                    return
                ps_y = yps.tile([_P, _QR], F, name="yps")
                sacc = sapool.tile([_P, _QR], R, name="sacc")
                nc.gpsimd.memset(sacc[:].bitcast(mybir.dt.uint32), 0)
                for i, kt in enumerate(kts):
                    cat, pidx, q_lo = plan.get((qr, kt), ("full", -1, 0))
                    sl = slice(q_lo, _QR)
                    ps_s = attps.tile([_P, _QR], F, name="sps")
                    nc.tensor.matmul(
                        ps_s[:, sl],
                        lhsT=KT[h][:, kt * _P:(kt + 1) * _P],
                        rhs=qtile[:, sl],
                        start=True,
                        stop=True,
                    )
                    pt = ptpool.tile([_P, _QR], VDT, name="pt")
                    nc.scalar.activation(pt[:, sl], ps_s[:, sl], Exp, scale=_SCALE)
                    if cat == "mixed":
                        nc.gpsimd.tensor_mul(
                            pt[:, sl], pt[:, sl], get_mask_tile(pidx)[:, sl]
                        )
                    nc.vector.tensor_add(sacc[:, sl], sacc[:, sl], pt[:, sl])
                    nc.tensor.matmul(
                        ps_y[:, sl],
                        lhsT=V[kt][:, h * _HD:(h + 1) * _HD],
                        rhs=pt[:, sl],
                        start=(i == 0),
                        stop=(i == len(kts) - 1),
                    )
                ps_bc = attps.tile([_P, _QR], F, name="sps")
                nc.tensor.matmul(
                    ps_bc[:], lhsT=ones_r[:], rhs=sacc[:], start=True, stop=True
                )
                rec = sapool.tile([_P, _QR], F, name="rec", bufs=1)
                nc.vector.reciprocal(rec[:], ps_bc[:])
                nc.vector.tensor_mul(ysb[:], ps_y[:], rec[:])
                y_tiles[(h, qr)] = ysb

            def C_phase(qr):
                for fr in range(_C // _QR):
                    wps = []
                    for h in range(_HL):
                        w = wppool.tile([_P, _QR], R, name="wp")
                        nc.sync.dma_start(
                            w[:],
                            wproj.ap()[
                                h * _HD:(h + 1) * _HD, fr * _QR:(fr + 1) * _QR
                            ].bitcast(R),
                        )
                        wps.append(w)
                    for tsub in range(_QR // _P):
                        ps = mmps.tile([_P, _QR], F, name="mmps")
                        for h in range(_HL):
                            nc.tensor.matmul(
                                ps[:],
                                lhsT=y_tiles[(h, qr)][:, tsub * _P:(tsub + 1) * _P],
                                rhs=wps[h][:],
                                start=(h == 0),
                                stop=(h == _HL - 1),
                            )
                        ot = opool.tile([_P, _QR], F, name="osb")
                        nc.any.tensor_copy(ot[:], ps[:])
                        t0 = qr * _QR + tsub * _P
                        nc.sync.dma_start(
                            out.ap()[t0:t0 + _P, fr * _QR:(fr + 1) * _QR], ot[:]
                        )

            def interleave_emit(primary, filler):
                """Emit primary units with filler spread evenly between them
                (emission order = Tile scheduling priority)."""
                if not primary:
                    for u in filler:
                        u()
                    return
                n, m = len(primary), len(filler)
                fi = 0
                for i, u in enumerate(primary):
                    u()
                    want = (m * (i + 1)) // n
                    while fi < want:
                        filler[fi]()
                        fi += 1
                while fi < m:
                    filler[fi]()
                    fi += 1

            def make_A_units(tr):
                return [lambda tr=tr: A_xload(tr)] + [
                    lambda tr=tr, f=f: A_fblock(tr, f) for f in range(NF)
                ]

            def make_BC_units(tr):
                units = []
                if "B" not in phases:
                    return units
                for sub in range(SUB):
                    qr = tr * SUB + sub
                    for h in range(_HL):
                        units.append(
                            lambda h=h, qr=qr: B_phase(h, qr, qt_tiles[(h, qr)])
                        )
                    if "C" in phases:
                        units.append(lambda qr=qr: C_phase(qr))
                return units

            for _rep in range(repeat):
                # interleaving does not cross repeat boundaries: A(rep+1)
                # rewrites KT/V which B(rep) still reads (WAR through
                # later-priority work deadlocks the queues)
                pending_bc = []
                for tr in range(NTR):
                    # A(tr) work is interleaved into the previous iteration's
                    # B/C units so the PE always has dense matmul work to
                    # hide the exp-latency-bound attention stretches (needs
                    # the double-size qt pool, so only in the bf16 layout)
                    if v_bf16:
                        interleave_emit(pending_bc, make_A_units(tr))
                    else:
                        for u in pending_bc:
                            u()
                        for u in make_A_units(tr):
                            u()
                    pending_bc = make_BC_units(tr)
                for u in pending_bc:
                    u()

    if split_waits:
        _split_excess_waits(nc)
    return nc


# ---------------------------------------------------------------------------
# Execution via PJRT (axon tunnel)
# ---------------------------------------------------------------------------

def _make_runner(nc, n_cores):
    import jax
    from concourse import bass2jax, mybir
    from jax.sharding import Mesh, PartitionSpec

    try:
        from jax.experimental.shard_map import shard_map
    except ImportError:
        from jax import shard_map  # newer jax

    bass2jax.install_neuronx_cc_hook()

    partition_name = nc.partition_id_tensor.name if nc.partition_id_tensor else None
    in_names, out_names, out_avals, zero_outs = [], [], [], []
    for alloc in nc.m.functions[0].allocations:
        if not isinstance(alloc, mybir.MemoryLocationSet):
            continue
        name = alloc.memorylocations[0].name
        if alloc.kind == "ExternalInput":
            if name != partition_name:
                in_names.append(name)
        elif alloc.kind == "ExternalOutput":
            out_names.append(name)
            shape = tuple(alloc.tensor_shape)
            dtype = mybir.dt.np(alloc.dtype)
            out_avals.append(jax.core.ShapedArray(shape, dtype))
            zero_outs.append(np.zeros(shape, dtype))
    n_params = len(in_names)
    all_names = list(in_names) + list(out_names)
    if partition_name is not None:
        all_names.append(partition_name)
    all_names = tuple(all_names)

    def _body(*args):
        operands = list(args)
        if partition_name is not None:
            operands.append(bass2jax.partition_id_tensor())
        outs = bass2jax._bass_exec_p.bind(
            *operands,
            out_avals=tuple(out_avals),
            in_names=all_names,
            out_names=tuple(out_names),
            lowering_input_output_aliases=(),
            sim_require_finite=True,
            sim_require_nnan=True,
            nc=nc,
        )
        return tuple(outs)

    devices = jax.devices()[:n_cores]
    mesh = Mesh(np.asarray(devices), ("core",))
    in_specs = (PartitionSpec("core"),) * (n_params + len(out_names))
    out_specs = (PartitionSpec("core"),) * len(out_names)
    donate = tuple(range(n_params, n_params + len(out_names)))
    fn = jax.jit(
        shard_map(
            _body, mesh=mesh, in_specs=in_specs, out_specs=out_specs, check_rep=False
        ),
        donate_argnums=donate,
        keep_unused=True,
    )
    return {
        "fn": fn,
        "mesh": mesh,
        "in_names": in_names,
        "out_names": out_names,
        "out_avals": out_avals,
        "zero_outs": zero_outs,
        "n_cores": n_cores,
    }


def _concat_inputs(runner, per_core_inmaps):
    return [
        np.concatenate([np.asarray(m[name]) for m in per_core_inmaps], axis=0)
        for name in runner["in_names"]
    ]


def _concat_zeros(runner):
    n = runner["n_cores"]
    return [
        np.zeros((n * z.shape[0], *z.shape[1:]), z.dtype) for z in runner["zero_outs"]
    ]


def _run_cores(runner, per_core_inmaps):
    outs = runner["fn"](*_concat_inputs(runner, per_core_inmaps), *_concat_zeros(runner))
    n = runner["n_cores"]
    res = [np.asarray(o) for o in outs]
    return [
        {
            name: res[i].reshape(n, *runner["out_avals"][i].shape)[c]
            for i, name in enumerate(runner["out_names"])
        }
        for c in range(n)
    ]


# ---------------------------------------------------------------------------
# Public entry point
# ---------------------------------------------------------------------------

def _get_runner(T_, plan, patterns):
    key = (
        T_,
        V_BF16,
        tuple(sorted((k, *v) for k, v in plan.items())),
        hashlib.sha1(patterns.tobytes()).hexdigest(),
    )
    if key not in _BUILD_CACHE:
        nc = _build_nc(T_, plan, patterns.shape[0], V_BF16)
        _BUILD_CACHE[key] = _make_runner(nc, _NCORES)
    return _BUILD_CACHE[key]


def _make_inmaps(x, mask_plan_patterns, W_qkv, W_proj):
    plan, patterns = mask_plan_patterns
    Bn, T_, C_ = x.shape
    aux = np.concatenate(
        [np.eye(_P, dtype=np.float32), np.ones((_P, _P), np.float32)], axis=1
    )
    in_maps = []
    xts = [np.ascontiguousarray(x[b].T) for b in range(Bn)]
    for core in range(_NCORES):
        b, g = divmod(core, _NG)
        wqkv_g = np.concatenate(
            [
                W_qkv[:, g * _DL:(g + 1) * _DL],
                W_qkv[:, C_ + g * _DL:C_ + (g + 1) * _DL],
                W_qkv[:, 2 * C_ + g * _DL:2 * C_ + (g + 1) * _DL],
            ],
            axis=1,
        )
        # rearrange to [NF, 128, CT*128] so each f-tile is one contiguous
        # per-partition DMA stream (see _build_nc)
        CT = C_ // _P
        NF = 3 * _HL
        wqkv_g = np.ascontiguousarray(
            wqkv_g.reshape(CT, _P, NF, _P).transpose(2, 1, 0, 3).reshape(NF, _P, C_)
        )
        wproj_g = np.ascontiguousarray(W_proj[g * _DL:(g + 1) * _DL, :])
        in_maps.append(
            {
                "xt": xts[b],
                "wqkv": wqkv_g,
                "wproj": wproj_g,
                "mpat": patterns,
                "aux": aux,
            }
        )
    return in_maps


def _combine(per_core, b_proj, Bn, T_, C_):
    outp = np.empty((Bn, T_, C_), np.float32)
    for b in range(Bn):
        acc = per_core[b * _NG]["out"].astype(np.float32, copy=True)
        for g in range(1, _NG):
            acc += per_core[b * _NG + g]["out"]
        outp[b] = acc + b_proj[None, :].astype(np.float32)
    return outp


def kernel(**inputs):
    x = np.asarray(inputs["x"], dtype=np.float32)
    Bn, T_, C_ = x.shape
    assert (Bn, C_) == (_B, _C), f"kernel hardcoded for B={_B}, C={_C}"
    mask = np.asarray(inputs["mask"]).astype(bool).reshape(T_, T_)
    W_qkv = np.asarray(inputs["W_qkv"], dtype=np.float32)
    W_proj = np.asarray(inputs["W_proj"], dtype=np.float32)
    b_proj = np.asarray(inputs["b_proj"], dtype=np.float32)

    plan, patterns = _analyze_mask(mask, T_)
    runner = _get_runner(T_, plan, patterns)
    in_maps = _make_inmaps(x, (plan, patterns), W_qkv, W_proj)
    per_core = _run_cores(runner, in_maps)
    return _combine(per_core, b_proj, Bn, T_, C_)
